# revision 21
# baseline (speedup 1.0000x reference)
"""Distributed causal self-attention for TRN2 (8 NeuronCores).

Sharding: tensor-parallel over heads (2 heads/core). Each core computes
q,k,v for its heads over the full sequence (column-sharded c_attn), runs
causal attention for them, reshards the attention output sequence-wise
with 4 chunked AllToAlls (overlapped with attention of later chunks), and
applies the full output projection to its 256 rows (row-sharded c_proj).

Row ownership is striped: within q-chunk qc (512 rows), rows
[512*qc + 64*j : 512*qc + 64*(j+1)] belong to core j. Core j's "out"
holds its 4 stripes in qc order; the host reassembles.

Compute dtype: bf16 operands, fp32 PSUM accumulation.

Per-core layouts (S=2048, E=1024, D=64, F=128 local feats):
  xt   (E, S)  bf16  x^T               wqkv (E, 3F) bf16  [Wq*s|Wk|Wv]^T
  bqkv (128,3) f32   bias columns      wpt  (E, E)  bf16  w_proj^T
  bp   (1, E)  bf16  b_proj            out  (256,E) f32
"""

import numpy as np
import ml_dtypes

import concourse.bass as bass
import concourse.mybir as mybir
import concourse.tile as tile
from concourse import bacc
from concourse.masks import make_identity, make_upper_triangular
from concourse.tile import add_dep_helper

S, E, H = 2048, 1024, 16
D = E // H          # 64 head dim
NCORES = 8
HPC = H // NCORES   # 2 heads per core
F = HPC * D         # 128 local features
SQ = S // NCORES    # 256 output rows per core
ST = 64             # per-core stripe within a q chunk
P = 128
QC = 512            # q chunk (columns per attention pass)
NQC = S // QC       # 4
NKB = S // P        # 16 k blocks
KCH = E // P        # 8 contraction chunks for E-dim matmuls

F32 = mybir.dt.float32
BF16 = mybir.dt.bfloat16
EXP = mybir.ActivationFunctionType.Exp


def build_nc():
    nc = bacc.Bacc("TRN2", target_bir_lowering=False, debug=False,
                   num_devices=NCORES, enable_partition_id=True)

    xt = nc.dram_tensor("xt", [E, S], BF16, kind="ExternalInput")
    wqkv = nc.dram_tensor("wqkv", [E, 3 * F], BF16, kind="ExternalInput")
    bqkv = nc.dram_tensor("bqkv", [P, 3], F32, kind="ExternalInput")
    wpt = nc.dram_tensor("wpt", [E, E], BF16, kind="ExternalInput")
    bp = nc.dram_tensor("bp", [1, E], BF16, kind="ExternalInput")
    out = nc.dram_tensor("out", [SQ, E], F32, kind="ExternalOutput")

    with tile.TileContext(nc) as tc:
        _body(nc, tc, xt, wqkv, bqkv, wpt, bp, out)

    nc.compile()
    return nc


def _body(nc, tc, xt, wqkv, bqkv, wpt, bp, out):
    import contextlib
    ctx = contextlib.ExitStack()
    with ctx:
        constp = ctx.enter_context(tc.tile_pool(name="constp", bufs=1))
        wqp = ctx.enter_context(tc.tile_pool(name="wqp", bufs=1))
        xtp = ctx.enter_context(tc.tile_pool(name="xtp", bufs=1))
        qkvp = ctx.enter_context(tc.tile_pool(name="qkvp", bufs=1))
        vop = ctx.enter_context(tc.tile_pool(name="vop", bufs=1))
        wptp = ctx.enter_context(tc.tile_pool(name="wptp", bufs=1))
        atp = ctx.enter_context(tc.tile_pool(name="atp", bufs=1))
        expp = ctx.enter_context(tc.tile_pool(name="expp", bufs=4))
        stagep = ctx.enter_context(tc.tile_pool(name="stagep", bufs=2))
        smallp = ctx.enter_context(tc.tile_pool(name="smallp", bufs=4))
        outp = ctx.enter_context(tc.tile_pool(name="outp", bufs=2))
        psmm = ctx.enter_context(tc.tile_pool(name="psmm", bufs=2, space="PSUM"))
        pslog = ctx.enter_context(tc.tile_pool(name="pslog", bufs=2, space="PSUM"))
        psav = ctx.enter_context(tc.tile_pool(name="psav", bufs=1, space="PSUM"))
        dramp = ctx.enter_context(tc.tile_pool(name="dramp", bufs=1, space="DRAM"))

        # ---- constants (built in f32, cast-copied to bf16) --------------
        ident_f = constp.tile([P, P], F32, name="ident_f")
        make_identity(nc, ident_f[:, :])
        ident = constp.tile([P, P], BF16, name="ident")
        nc.vector.tensor_copy(ident[:, :], ident_f[:, :])
        tri_f = constp.tile([P, P], F32, name="tri_f")  # tri[k,q] = 1 if q >= k
        make_upper_triangular(nc, tri_f[:, :], val=1.0, diag=True)
        tri = constp.tile([P, P], BF16, name="tri")
        nc.vector.tensor_copy(tri[:, :], tri_f[:, :])
        ones_f = constp.tile([P, 1], F32, name="ones_f")
        nc.vector.memset(ones_f[:, :], 1.0)
        ones1 = constp.tile([1, P], BF16, name="ones1")
        nc.vector.tensor_copy(ones1[:, :], ones_f[0:1, 0:1].to_broadcast((1, P)))

        bq_sb = constp.tile([P, 3], F32, name="bq_sb")
        nc.sync.dma_start(bq_sb[:, :], bqkv[:, :])

        # ---- weight + activation loads (per contraction chunk) ----------
        wq_sb = [wqp.tile([P, 3 * F], BF16, name=f"wq_sb{k}", tag=f"wq{k}")
                 for k in range(KCH)]
        xt_sb = [xtp.tile([P, S], BF16, name=f"xt_sb{k}", tag=f"xt{k}")
                 for k in range(KCH)]
        # split across the two HWDGE queues (sync + scalar)
        for k in range(KCH):
            eng = nc.sync if k % 2 == 0 else nc.scalar
            eng.dma_start(wq_sb[k][:, :], wqkv[k * P:(k + 1) * P, :])
            eng.dma_start(xt_sb[k][:, :], xt[k * P:(k + 1) * P, :])

        qkv_sb = [qkvp.tile([P, 3, QC], BF16, name=f"qkv_sb{n}", tag=f"qkv{n}")
                  for n in range(NQC)]
        vones = [vop.tile([P, 2 * (D + 1)], BF16, name=f"vones{kb}",
                          tag=f"vo{kb}") for kb in range(NKB)]

        # reshard groups (out row order stays qc 1,2,3,0): all-gather the
        # head-sharded attention output, then each core slices its own row
        # stripes out of the gathered buffer with a partition-id offset
        GROUPS = [[1, 2], [3, 0]]
        NG = len(GROUPS)
        ag_in = [dramp.tile([F, len(GROUPS[g]) * QC], BF16,
                            name=f"ag_in{g}", tag=f"ai{g}")
                 for g in range(NG)]
        ag_out = [dramp.tile([NCORES * F, len(GROUPS[g]) * QC], BF16,
                             name=f"ag_out{g}", tag=f"ao{g}",
                             addr_space="Shared")
                  for g in range(NG)]
        at_sb = [atp.tile([P, KCH, len(GROUPS[g]) * ST], BF16,
                          name=f"at_sb{g}", tag=f"at{g}") for g in range(NG)]
        wp_sb = wptp.tile([P, KCH, E], BF16, name="wp_sb")
        bp_sb = constp.tile([1, E], BF16, name="bp_sb")

        def emit_qkv_m(n, m):
            nsl = slice(n * QC, (n + 1) * QC)
            pt = psmm.tile([P, QC], F32, tag="mmp", name="qkv_ps")
            for k in range(KCH):
                nc.tensor.matmul(
                    pt[:, :], lhsT=wq_sb[k][:, m * P:(m + 1) * P],
                    rhs=xt_sb[k][:, nsl],
                    start=(k == 0), stop=(k == KCH - 1))
            nc.vector.tensor_add(
                qkv_sb[n][:, m, :], pt[:, :],
                bq_sb[:, m:m + 1].to_broadcast((P, QC)))

        def emit_qkv(n):
            for m in range(3):
                emit_qkv_m(n, m)

        def emit_vtrans(kb):
            n = kb // 4
            tp = psmm.tile([P, QC], BF16, tag="mmp", name="vt_ps")
            nc.tensor.transpose(
                tp[:, :P], qkv_sb[n][:, 2, (kb % 4) * P:(kb % 4 + 1) * P],
                ident[:, :])
            vo = vones[kb]
            nc.vector.tensor_copy(vo[:, 0:D], tp[:, 0:D])
            nc.vector.tensor_copy(vo[:, D + 1:2 * D + 1], tp[:, D:2 * D])
            nc.vector.tensor_copy(vo[:, D:D + 1], ones_f[:, :])
            nc.vector.tensor_copy(vo[:, 2 * D + 1:2 * D + 2], ones_f[:, :])

        def emit_attn(qc, fillers=()):
            # fillers: emission callables sprinkled between k blocks so the
            # PE keeps independent work queued while exp stalls attention
            fillers = list(fillers)
            nkb = 4 * qc + 4
            avp = [psav.tile([D + 1, QC], F32, tag=f"avp{h}",
                             name=f"av_ps{h}") for h in range(HPC)]
            pend = []  # deferred attn@v (2 k blocks deep)

            def flush(item, last):
                kb, et, qoff, N = item
                mm = None
                for h in range(HPC):
                    mm = nc.tensor.matmul(
                        avp[h][:, qoff:QC],
                        lhsT=vones[kb][:, h * (D + 1):(h + 1) * (D + 1)],
                        rhs=et[:, h, :N],
                        start=(kb == 0), stop=last)
                return mm

            for kb in range(nkb):
                diag = kb >= 4 * qc
                qoff = P * (kb - 4 * qc) if diag else 0
                N = QC - qoff
                qsl = slice(qc * QC + qoff, (qc + 1) * QC)
                lqsl = slice(qoff, QC)
                # two heads' logits into the two banks of one psum tile
                lp = pslog.tile([P, 2 * QC], F32, tag="logp", name="log_ps")
                for h in range(HPC):
                    nc.tensor.matmul(
                        lp[:, h * QC:h * QC + N],
                        lhsT=qkv_sb[kb // 4][D * h:D * (h + 1), 1,
                                             (kb % 4) * P:(kb % 4 + 1) * P],
                        rhs=qkv_sb[qc][D * h:D * (h + 1), 0, lqsl],
                        start=True, stop=True)
                et = expp.tile([P, 2, QC], BF16, tag="et", name="exp_sb")
                nc.scalar.activation(
                    et[:, :, :N],
                    lp[:, :].rearrange("p (b n) -> p b n", b=2)[:, :, :N],
                    EXP)
                if diag:
                    nc.vector.tensor_mul(
                        et[:, :, 0:P], et[:, :, 0:P],
                        tri[:, None, :].to_broadcast((P, 2, P)))
                if len(pend) >= 2:
                    flush(pend.pop(0), False)
                pend.append((kb, et, qoff, N))
                if fillers and kb % 2 == 1:
                    fillers.pop(0)()
            for f in fillers:
                f()
            last_av = None
            while pend:
                last_av = flush(pend.pop(0), not pend)

            # normalize rows 0:64 by row 64 (exp sums), both heads into one
            # staging tile, then scatter stripes into the a2a input buffer
            g, slot = next((g, s) for g in range(NG)
                           for s in range(len(GROUPS[g])) if GROUPS[g][s] == qc)
            stage = stagep.tile([P, QC], BF16, tag="stage", name="stage")
            for h in range(HPC):
                rs = smallp.tile([1, QC], F32, tag="rs", name="rs")
                nc.vector.tensor_copy(rs[:, :], avp[h][D:D + 1, :])
                rr = smallp.tile([1, QC], F32, tag="rr", name="rr")
                nc.vector.reciprocal_approx_fast(rr[:, :], rs[:, :])
                rb = smallp.tile([D, QC], F32, tag="rb", name="rb")
                nc.gpsimd.partition_broadcast(rb[:, :], rr[:, :])
                nc.vector.tensor_mul(
                    stage[D * h:D * (h + 1), :], avp[h][0:D, :], rb[:, :])
            nc.sync.dma_start(ag_in[g][:, slot * QC:(slot + 1) * QC],
                              stage[:, :])
            if slot == len(GROUPS[g]) - 1:
                nc.gpsimd.collective_compute(
                    "AllGather", mybir.AluOpType.bypass,
                    replica_groups=[list(range(NCORES))],
                    ins=[ag_in[g][:, :].opt()],
                    outs=[ag_out[g][:, :].opt()])
            return last_av

        def emit_proj(ph, anchor):
            # projection for stripe pair (2*ph, 2*ph+1): 128 output rows.
            # anchor: keep these instructions behind the attention stream in
            # the static schedule - the scheduler otherwise hoists them and
            # the collective-gated loads head-of-line block the PE/DMA FIFOs.
            def pin(inst):
                if anchor is not None:
                    add_dep_helper(inst.ins, anchor.ins, sync=False,
                                   reason="proj after attention")
                return inst
            # stripe s (0..3, out-row order) lives in group g at slot
            def stripe_src(s, k):
                acc = 0
                for g in range(NG):
                    if s < acc + len(GROUPS[g]):
                        slot = s - acc
                        return at_sb[g][:, k, slot * ST:(slot + 1) * ST]
                    acc += len(GROUPS[g])
            for n in range(E // QC):
                nsl = slice(n * QC, (n + 1) * QC)
                pp = psmm.tile([P, QC], F32, tag="mmp", name="proj_ps")
                for k in range(KCH):
                    for half in range(2):
                        pin(nc.tensor.matmul(
                            pp[half * ST:(half + 1) * ST, :],
                            lhsT=stripe_src(2 * ph + half, k),
                            rhs=wp_sb[:, k, nsl],
                            start=(k == 0), stop=False,
                            tile_position=(0, half * ST)))
                pin(nc.tensor.matmul(
                    pp[:, :], lhsT=ones1[:, :], rhs=bp_sb[:, nsl],
                    start=False, stop=True))
                ob = outp.tile([P, QC], F32, tag="ob", name="ob")
                nc.vector.tensor_copy(ob[:, :], pp[:, :])
                nc.sync.dma_start(out[ph * P:(ph + 1) * P, nsl], ob[:, :])

        # ---- emission: interleave qkv chunks with attention so both PE
        # phases and the ACT exp stream overlap; attention runs 1,2,3,0 so
        # the final collective only waits on the shortest chunk (qc0).
        emit_qkv(0)
        for kb in range(0, 4):
            emit_vtrans(kb)
        emit_qkv(1)
        for kb in range(4, 8):
            emit_vtrans(kb)
        for k in range(KCH):
            eng = nc.sync if k % 2 == 0 else nc.scalar
            eng.dma_start(wp_sb[:, k, :], wpt[k * P:(k + 1) * P, :])
        nc.sync.dma_start(bp_sb[:, :], bp[:, :])
        f1 = [lambda m=m: emit_qkv_m(2, m) for m in range(3)] + \
             [lambda kb=kb: emit_vtrans(kb) for kb in range(8, 12)]
        emit_attn(1, f1)
        f2 = [lambda m=m: emit_qkv_m(3, m) for m in range(3)] + \
             [lambda kb=kb: emit_vtrans(kb) for kb in range(12, 16)]
        emit_attn(2, f2)
        emit_attn(3)
        last_av = emit_attn(0)
        pid = nc.partition_id()
        for g in range(NG):
            gq = a2a = ag_out[g][:, :].rearrange("(kc p) c -> p kc c", p=P)
            for slot in range(len(GROUPS[g])):
                d = nc.sync.dma_start(
                    at_sb[g][:, :, slot * ST:(slot + 1) * ST],
                    gq[:, :, bass.ds(slot * QC + ST * pid, ST)])
                add_dep_helper(d.ins, last_av.ins, sync=False,
                               reason="gathered loads after attention")
        for ph in range(2):
            emit_proj(ph, last_av)


_NC_CACHE = None


def _get_nc():
    global _NC_CACHE
    if _NC_CACHE is None:
        _NC_CACHE = build_nc()
    return _NC_CACHE


def make_in_maps(x, w_attn, b_attn, w_proj, b_proj):
    bf16 = ml_dtypes.bfloat16
    x = np.asarray(x, dtype=np.float32)
    w_attn = np.asarray(w_attn, dtype=np.float32)
    b_attn = np.asarray(b_attn, dtype=np.float32)
    w_proj = np.asarray(w_proj, dtype=np.float32)
    b_proj = np.asarray(b_proj, dtype=np.float32)

    xt = np.ascontiguousarray(x.T).astype(bf16)          # (E, S)
    wpt = np.ascontiguousarray(w_proj.T).astype(bf16)    # (E, E)
    bpa = np.ascontiguousarray(b_proj[None, :]).astype(bf16)
    scale = 1.0 / np.sqrt(D)

    in_maps = []
    for c in range(NCORES):
        rq = slice(F * c, F * (c + 1))
        rk = slice(E + F * c, E + F * (c + 1))
        rv = slice(2 * E + F * c, 2 * E + F * (c + 1))
        wqkv = np.ascontiguousarray(np.concatenate(
            [w_attn[rq] * scale, w_attn[rk], w_attn[rv]], axis=0).T)
        bq = np.stack([b_attn[rq] * scale, b_attn[rk], b_attn[rv]], axis=1)
        in_maps.append({
            "xt": xt,
            "wqkv": wqkv.astype(bf16),
            "bqkv": np.ascontiguousarray(bq, dtype=np.float32),
            "wpt": wpt,
            "bp": bpa,
        })
    return in_maps


def run(inputs, trace=False, **kw):
    from concourse.bass_utils import run_bass_kernel_spmd
    nc = _get_nc()
    in_maps = make_in_maps(**inputs)
    res = run_bass_kernel_spmd(nc, in_maps, core_ids=list(range(NCORES)),
                               trace=trace, **kw)
    # core j's out row blocks are stripes for qc = 3,2,1,0 in that order;
    # stripe qc covers global rows 512*qc + 64*j .. +64
    full = np.empty((S, E), dtype=np.float32)
    for j in range(NCORES):
        o = res.results[j]["out"]                        # (256, E)
        for blk, qc in enumerate([1, 2, 3, 0]):
            full[QC * qc + ST * j: QC * qc + ST * (j + 1), :] = \
                o[ST * blk: ST * (blk + 1), :]
    return full, res


def kernel(**inputs):
    full, _ = run(inputs, trace=False)
    return full


# revision 28
# speedup vs baseline: 1.1179x; 1.1179x over previous
"""Distributed causal self-attention for TRN2 (8 NeuronCores).

Sharding: tensor-parallel over heads (2 heads/core). Each core computes
q,k,v for its heads over the full sequence (column-sharded c_attn), runs
causal attention for them, reshards the attention output sequence-wise
with 4 chunked AllToAlls (overlapped with attention of later chunks), and
applies the full output projection to its 256 rows (row-sharded c_proj).

Row ownership is striped: within q-chunk qc (512 rows), rows
[512*qc + 64*j : 512*qc + 64*(j+1)] belong to core j. Core j's "out"
holds its 4 stripes in qc order; the host reassembles.

Compute dtype: bf16 operands, fp32 PSUM accumulation.

Per-core layouts (S=2048, E=1024, D=64, F=128 local feats):
  xt   (E, S)  bf16  x^T               wqkv (E, 3F) bf16  [Wq*s|Wk|Wv]^T
  bqkv (128,3) f32   bias columns      wpt  (E, E)  bf16  w_proj^T
  bp   (1, E)  bf16  b_proj            out  (256,E) f32
"""

import numpy as np
import ml_dtypes

import concourse.bass as bass
import concourse.mybir as mybir
import concourse.tile as tile
from concourse import bacc
from concourse.masks import make_identity, make_upper_triangular
from concourse.tile import add_dep_helper

S, E, H = 2048, 1024, 16
D = E // H          # 64 head dim
NCORES = 8
HPC = H // NCORES   # 2 heads per core
F = HPC * D         # 128 local features
SQ = S // NCORES    # 256 output rows per core
ST = 64             # per-core stripe within a q chunk
P = 128
QC = 512            # q chunk (columns per attention pass)
NQC = S // QC       # 4
NKB = S // P        # 16 k blocks
KCH = E // P        # 8 contraction chunks for E-dim matmuls

F32 = mybir.dt.float32
BF16 = mybir.dt.bfloat16
EXP = mybir.ActivationFunctionType.Exp


def build_nc():
    nc = bacc.Bacc("TRN2", target_bir_lowering=False, debug=False,
                   num_devices=NCORES, enable_partition_id=True)

    xt = nc.dram_tensor("xt", [E, S], BF16, kind="ExternalInput")
    wqkv = nc.dram_tensor("wqkv", [E, 3 * F], BF16, kind="ExternalInput")
    bqkv = nc.dram_tensor("bqkv", [P, 3], F32, kind="ExternalInput")
    wpt = nc.dram_tensor("wpt", [E, E], BF16, kind="ExternalInput")
    bp = nc.dram_tensor("bp", [1, E], BF16, kind="ExternalInput")
    out = nc.dram_tensor("out", [SQ, E], F32, kind="ExternalOutput")

    with tile.TileContext(nc) as tc:
        _body(nc, tc, xt, wqkv, bqkv, wpt, bp, out)

    nc.compile()
    return nc


def _body(nc, tc, xt, wqkv, bqkv, wpt, bp, out):
    import contextlib
    ctx = contextlib.ExitStack()
    with ctx:
        constp = ctx.enter_context(tc.tile_pool(name="constp", bufs=1))
        wqp = ctx.enter_context(tc.tile_pool(name="wqp", bufs=1))
        xtp = ctx.enter_context(tc.tile_pool(name="xtp", bufs=1))
        qkvp = ctx.enter_context(tc.tile_pool(name="qkvp", bufs=1))
        vop = ctx.enter_context(tc.tile_pool(name="vop", bufs=1))
        wptp = ctx.enter_context(tc.tile_pool(name="wptp", bufs=1))
        atp = ctx.enter_context(tc.tile_pool(name="atp", bufs=1))
        expp = ctx.enter_context(tc.tile_pool(name="expp", bufs=4))
        stagep = ctx.enter_context(tc.tile_pool(name="stagep", bufs=2))
        smallp = ctx.enter_context(tc.tile_pool(name="smallp", bufs=4))
        outp = ctx.enter_context(tc.tile_pool(name="outp", bufs=2))
        psmm = ctx.enter_context(tc.tile_pool(name="psmm", bufs=2, space="PSUM"))
        pslog = ctx.enter_context(tc.tile_pool(name="pslog", bufs=2, space="PSUM"))
        psav = ctx.enter_context(tc.tile_pool(name="psav", bufs=1, space="PSUM"))
        dramp = ctx.enter_context(tc.tile_pool(name="dramp", bufs=1, space="DRAM"))

        # ---- constants (built in f32, cast-copied to bf16) --------------
        ident_f = constp.tile([P, P], F32, name="ident_f")
        make_identity(nc, ident_f[:, :])
        ident = constp.tile([P, P], BF16, name="ident")
        nc.vector.tensor_copy(ident[:, :], ident_f[:, :])
        tri_f = constp.tile([P, P], F32, name="tri_f")  # tri[k,q] = 1 if q >= k
        make_upper_triangular(nc, tri_f[:, :], val=1.0, diag=True)
        tri = constp.tile([P, P], BF16, name="tri")
        nc.vector.tensor_copy(tri[:, :], tri_f[:, :])
        ones_f = constp.tile([P, 1], F32, name="ones_f")
        nc.vector.memset(ones_f[:, :], 1.0)
        ones_r = constp.tile([1, D], F32, name="ones_r")
        nc.vector.memset(ones_r[:, :], 1.0)
        ones1 = constp.tile([1, P], BF16, name="ones1")
        nc.vector.tensor_copy(ones1[:, :], ones_f[0:1, 0:1].to_broadcast((1, P)))

        bq_sb = constp.tile([P, 3], F32, name="bq_sb")
        nc.sync.dma_start(bq_sb[:, :], bqkv[:, :])

        # dependency-free warm-up matmuls: keep the PE busy through the HAM
        # activity window while the input DMAs are in flight, so the real
        # matmuls start at 2.4 GHz instead of 1.2
        warm = constp.tile([P, QC], BF16, name="warm")
        nc.vector.memset(warm[:, :], 0.0)
        for _ in range(18):
            wp_ps = psmm.tile([P, QC], F32, tag="mmp", name="warm_ps")
            nc.tensor.matmul(wp_ps[:, :], lhsT=warm[:, 0:P], rhs=warm[:, :],
                             start=True, stop=True)

        # ---- weight + activation loads (per contraction chunk) ----------
        wq_sb = [wqp.tile([P, 3 * F], BF16, name=f"wq_sb{k}", tag=f"wq{k}")
                 for k in range(KCH)]
        xt_sb = [xtp.tile([P, S], BF16, name=f"xt_sb{k}", tag=f"xt{k}")
                 for k in range(KCH)]
        # split across the two HWDGE queues (sync + scalar)
        for k in range(KCH):
            eng = nc.sync if k % 2 == 0 else nc.scalar
            eng.dma_start(wq_sb[k][:, :], wqkv[k * P:(k + 1) * P, :])
            eng.dma_start(xt_sb[k][:, :], xt[k * P:(k + 1) * P, :])

        qkv_sb = [qkvp.tile([P, 3, QC], BF16, name=f"qkv_sb{n}", tag=f"qkv{n}")
                  for n in range(NQC)]
        vones = [vop.tile([P, 2 * (D + 1)], BF16, name=f"vones{kb}",
                          tag=f"vo{kb}") for kb in range(NKB)]

        # a2a groups (out row order stays qc 1,2,3,0)
        GROUPS = [[1, 2], [3, 0]]
        NG = len(GROUPS)
        a2a_in = [dramp.tile([NCORES * F, len(GROUPS[g]) * ST], BF16,
                             name=f"a2a_in{g}", tag=f"ai{g}")
                  for g in range(NG)]
        a2a_out = [dramp.tile([NCORES * F, len(GROUPS[g]) * ST], BF16,
                              name=f"a2a_out{g}", tag=f"ao{g}")
                   for g in range(NG)]
        at_sb = [atp.tile([P, KCH, len(GROUPS[g]) * ST], BF16,
                          name=f"at_sb{g}", tag=f"at{g}") for g in range(NG)]
        wp_sb = wptp.tile([P, KCH, E], BF16, name="wp_sb")
        bp_sb = constp.tile([1, E], BF16, name="bp_sb")

        def emit_qkv_m(n, m):
            nsl = slice(n * QC, (n + 1) * QC)
            pt = psmm.tile([P, QC], F32, tag="mmp", name="qkv_ps")
            for k in range(KCH):
                nc.tensor.matmul(
                    pt[:, :], lhsT=wq_sb[k][:, m * P:(m + 1) * P],
                    rhs=xt_sb[k][:, nsl],
                    start=(k == 0), stop=(k == KCH - 1))
            nc.vector.tensor_add(
                qkv_sb[n][:, m, :], pt[:, :],
                bq_sb[:, m:m + 1].to_broadcast((P, QC)))

        def emit_qkv(n):
            for m in range(3):
                emit_qkv_m(n, m)

        def emit_vtrans(kb):
            n = kb // 4
            tp = psmm.tile([P, QC], BF16, tag="mmp", name="vt_ps")
            nc.tensor.transpose(
                tp[:, :P], qkv_sb[n][:, 2, (kb % 4) * P:(kb % 4 + 1) * P],
                ident[:, :])
            vo = vones[kb]
            nc.vector.tensor_copy(vo[:, 0:D], tp[:, 0:D])
            nc.vector.tensor_copy(vo[:, D + 1:2 * D + 1], tp[:, D:2 * D])
            nc.vector.tensor_copy(vo[:, D:D + 1], ones_f[:, :])
            nc.vector.tensor_copy(vo[:, 2 * D + 1:2 * D + 2], ones_f[:, :])

        def emit_attn(qc, fillers=()):
            # fillers: emission callables sprinkled between k blocks so the
            # PE keeps independent work queued while exp stalls attention
            fillers = list(fillers)
            nkb = 4 * qc + 4
            avp = [psav.tile([D + 1, QC], F32, tag=f"avp{h}",
                             name=f"av_ps{h}") for h in range(HPC)]
            pend = []  # deferred attn@v (2 k blocks deep)

            def flush(item, last):
                kb, et, qoff, N = item
                mm = None
                for h in range(HPC):
                    mm = nc.tensor.matmul(
                        avp[h][:, qoff:QC],
                        lhsT=vones[kb][:, h * (D + 1):(h + 1) * (D + 1)],
                        rhs=et[:, h, :N],
                        start=(kb == 0), stop=last)
                return mm

            for kb in range(nkb):
                diag = kb >= 4 * qc
                qoff = P * (kb - 4 * qc) if diag else 0
                N = QC - qoff
                qsl = slice(qc * QC + qoff, (qc + 1) * QC)
                lqsl = slice(qoff, QC)
                # two heads' logits into the two banks of one psum tile
                lp = pslog.tile([P, 2 * QC], F32, tag="logp", name="log_ps")
                for h in range(HPC):
                    nc.tensor.matmul(
                        lp[:, h * QC:h * QC + N],
                        lhsT=qkv_sb[kb // 4][D * h:D * (h + 1), 1,
                                             (kb % 4) * P:(kb % 4 + 1) * P],
                        rhs=qkv_sb[qc][D * h:D * (h + 1), 0, lqsl],
                        start=True, stop=True)
                et = expp.tile([P, 2, QC], BF16, tag="et", name="exp_sb")
                nc.scalar.activation(
                    et[:, :, :N],
                    lp[:, :].rearrange("p (b n) -> p b n", b=2)[:, :, :N],
                    EXP)
                if diag:
                    nc.vector.tensor_mul(
                        et[:, :, 0:P], et[:, :, 0:P],
                        tri[:, None, :].to_broadcast((P, 2, P)))
                if len(pend) >= 2:
                    flush(pend.pop(0), False)
                pend.append((kb, et, qoff, N))
                if fillers and kb % 2 == 1:
                    fillers.pop(0)()
            for f in fillers:
                f()
            last_av = None
            while pend:
                last_av = flush(pend.pop(0), not pend)

            # normalize rows 0:64 by row 64 (exp sums), both heads into one
            # staging tile, then scatter stripes into the a2a input buffer
            g, slot = next((g, s) for g in range(NG)
                           for s in range(len(GROUPS[g])) if GROUPS[g][s] == qc)
            stage = stagep.tile([P, QC], BF16, tag="stage", name="stage")
            for h in range(HPC):
                rs = smallp.tile([1, QC], F32, tag="rs", name="rs")
                nc.vector.tensor_copy(rs[:, :], avp[h][D:D + 1, :])
                rr = smallp.tile([1, QC], F32, tag="rr", name="rr")
                nc.vector.reciprocal_approx_fast(rr[:, :], rs[:, :])
                # broadcast 1/sum across the 64 feature rows via a rank-1
                # matmul (cheaper + keeps gpsimd clear of the cc trigger path)
                rbp = psmm.tile([P, QC], F32, tag="mmp", name="rb_ps")
                nc.tensor.matmul(rbp[0:D, :], lhsT=ones_r[:, :], rhs=rr[:, :],
                                 start=True, stop=True)
                raw = smallp.tile([D, QC], F32, tag="rb", name="raw")
                nc.vector.tensor_copy(raw[:, :], avp[h][0:D, :])
                nc.vector.tensor_mul(
                    stage[D * h:D * (h + 1), :], raw[:, :], rbp[0:D, :])
            nc.sync.dma_start(
                a2a_in[g][:, :].rearrange("(j r) q -> r j q", r=P)
                [:, :, slot * ST:(slot + 1) * ST],
                stage[:, :].rearrange("p (j q) -> p j q", q=ST))
            if slot == len(GROUPS[g]) - 1:
                nc.gpsimd.collective_compute(
                    "AllToAll", mybir.AluOpType.bypass,
                    replica_groups=[list(range(NCORES))],
                    ins=[a2a_in[g][:, :].opt()],
                    outs=[a2a_out[g][:, :].opt()])
            return last_av

        def emit_proj(ph, anchor):
            # projection for stripe pair (2*ph, 2*ph+1): 128 output rows.
            # anchor: keep these instructions behind the attention stream in
            # the static schedule - the scheduler otherwise hoists them and
            # the collective-gated loads head-of-line block the PE/DMA FIFOs.
            def pin(inst):
                if anchor is not None:
                    add_dep_helper(inst.ins, anchor.ins, sync=False,
                                   reason="proj after attention")
                return inst
            # stripe s (0..3, out-row order) lives in group g at slot
            def stripe_src(s, k):
                acc = 0
                for g in range(NG):
                    if s < acc + len(GROUPS[g]):
                        slot = s - acc
                        return at_sb[g][:, k, slot * ST:(slot + 1) * ST]
                    acc += len(GROUPS[g])
            for n in range(E // QC):
                nsl = slice(n * QC, (n + 1) * QC)
                pp = psmm.tile([P, QC], F32, tag="mmp", name="proj_ps")
                for k in range(KCH):
                    for half in range(2):
                        pin(nc.tensor.matmul(
                            pp[half * ST:(half + 1) * ST, :],
                            lhsT=stripe_src(2 * ph + half, k),
                            rhs=wp_sb[:, k, nsl],
                            start=(k == 0), stop=False,
                            tile_position=(0, half * ST)))
                pin(nc.tensor.matmul(
                    pp[:, :], lhsT=ones1[:, :], rhs=bp_sb[:, nsl],
                    start=False, stop=True))
                ob = outp.tile([P, QC], F32, tag="ob", name="ob")
                nc.vector.tensor_copy(ob[:, :], pp[:, :])
                nc.sync.dma_start(out[ph * P:(ph + 1) * P, nsl], ob[:, :])

        # ---- emission: interleave qkv chunks with attention so both PE
        # phases and the ACT exp stream overlap; attention runs 1,2,3,0 so
        # the final collective only waits on the shortest chunk (qc0).
        emit_qkv(0)
        for kb in range(0, 4):
            emit_vtrans(kb)
        emit_qkv(1)
        for kb in range(4, 8):
            emit_vtrans(kb)
        for k in range(KCH):
            eng = nc.sync if k % 2 == 0 else nc.scalar
            eng.dma_start(wp_sb[:, k, :], wpt[k * P:(k + 1) * P, :])
        nc.sync.dma_start(bp_sb[:, :], bp[:, :])
        f1 = [lambda m=m: emit_qkv_m(2, m) for m in range(3)] + \
             [lambda kb=kb: emit_vtrans(kb) for kb in range(8, 12)]
        emit_attn(1, f1)
        f2 = [lambda m=m: emit_qkv_m(3, m) for m in range(3)] + \
             [lambda kb=kb: emit_vtrans(kb) for kb in range(12, 16)]
        emit_attn(2, f2)
        emit_attn(3)
        last_av = emit_attn(0)
        for g in range(NG):
            d = nc.sync.dma_start(
                at_sb[g][:, :, :],
                a2a_out[g][:, :].rearrange("(kc p) q -> p kc q", p=P))
            add_dep_helper(d.ins, last_av.ins, sync=False,
                           reason="gathered loads after attention")
        for ph in range(2):
            emit_proj(ph, last_av)


_NC_CACHE = None


def _get_nc():
    global _NC_CACHE
    if _NC_CACHE is None:
        _NC_CACHE = build_nc()
    return _NC_CACHE


def make_in_maps(x, w_attn, b_attn, w_proj, b_proj):
    bf16 = ml_dtypes.bfloat16
    x = np.asarray(x, dtype=np.float32)
    w_attn = np.asarray(w_attn, dtype=np.float32)
    b_attn = np.asarray(b_attn, dtype=np.float32)
    w_proj = np.asarray(w_proj, dtype=np.float32)
    b_proj = np.asarray(b_proj, dtype=np.float32)

    xt = np.ascontiguousarray(x.T).astype(bf16)          # (E, S)
    wpt = np.ascontiguousarray(w_proj.T).astype(bf16)    # (E, E)
    bpa = np.ascontiguousarray(b_proj[None, :]).astype(bf16)
    scale = 1.0 / np.sqrt(D)

    in_maps = []
    for c in range(NCORES):
        rq = slice(F * c, F * (c + 1))
        rk = slice(E + F * c, E + F * (c + 1))
        rv = slice(2 * E + F * c, 2 * E + F * (c + 1))
        wqkv = np.ascontiguousarray(np.concatenate(
            [w_attn[rq] * scale, w_attn[rk], w_attn[rv]], axis=0).T)
        bq = np.stack([b_attn[rq] * scale, b_attn[rk], b_attn[rv]], axis=1)
        in_maps.append({
            "xt": xt,
            "wqkv": wqkv.astype(bf16),
            "bqkv": np.ascontiguousarray(bq, dtype=np.float32),
            "wpt": wpt,
            "bp": bpa,
        })
    return in_maps


def run(inputs, trace=False, **kw):
    from concourse.bass_utils import run_bass_kernel_spmd
    nc = _get_nc()
    in_maps = make_in_maps(**inputs)
    res = run_bass_kernel_spmd(nc, in_maps, core_ids=list(range(NCORES)),
                               trace=trace, **kw)
    # core j's out row blocks are stripes for qc = 3,2,1,0 in that order;
    # stripe qc covers global rows 512*qc + 64*j .. +64
    full = np.empty((S, E), dtype=np.float32)
    for j in range(NCORES):
        o = res.results[j]["out"]                        # (256, E)
        for blk, qc in enumerate([1, 2, 3, 0]):
            full[QC * qc + ST * j: QC * qc + ST * (j + 1), :] = \
                o[ST * blk: ST * (blk + 1), :]
    return full, res


def kernel(**inputs):
    full, _ = run(inputs, trace=False)
    return full


# revision 30
# speedup vs baseline: 1.1295x; 1.0103x over previous
"""Distributed causal self-attention for TRN2 (8 NeuronCores).

Sharding: tensor-parallel over heads (2 heads/core). Each core computes
q,k,v for its heads over the full sequence (column-sharded c_attn), runs
causal attention for them, reshards the attention output sequence-wise
with 4 chunked AllToAlls (overlapped with attention of later chunks), and
applies the full output projection to its 256 rows (row-sharded c_proj).

Row ownership is striped: within q-chunk qc (512 rows), rows
[512*qc + 64*j : 512*qc + 64*(j+1)] belong to core j. Core j's "out"
holds its 4 stripes in qc order; the host reassembles.

Compute dtype: bf16 operands, fp32 PSUM accumulation.

Per-core layouts (S=2048, E=1024, D=64, F=128 local feats):
  xt   (E, S)  bf16  x^T               wqkv (E, 3F) bf16  [Wq*s|Wk|Wv]^T
  bqkv (128,3) f32   bias columns      wpt  (E, E)  bf16  w_proj^T
  bp   (1, E)  bf16  b_proj            out  (256,E) f32
"""

import numpy as np
import ml_dtypes

import concourse.bass as bass
import concourse.mybir as mybir
import concourse.tile as tile
from concourse import bacc
from concourse.masks import make_identity, make_upper_triangular
from concourse.tile import add_dep_helper

S, E, H = 2048, 1024, 16
D = E // H          # 64 head dim
NCORES = 8
HPC = H // NCORES   # 2 heads per core
F = HPC * D         # 128 local features
SQ = S // NCORES    # 256 output rows per core
ST = 64             # per-core stripe within a q chunk
P = 128
QC = 512            # q chunk (columns per attention pass)
NQC = S // QC       # 4
NKB = S // P        # 16 k blocks
KCH = E // P        # 8 contraction chunks for E-dim matmuls

F32 = mybir.dt.float32
BF16 = mybir.dt.bfloat16
EXP = mybir.ActivationFunctionType.Exp


def build_nc():
    nc = bacc.Bacc("TRN2", target_bir_lowering=False, debug=False,
                   num_devices=NCORES, enable_partition_id=True)

    xt = nc.dram_tensor("xt", [E, S], BF16, kind="ExternalInput")
    wqkv = nc.dram_tensor("wqkv", [E, 3 * F], BF16, kind="ExternalInput")
    bqkv = nc.dram_tensor("bqkv", [P, 3], F32, kind="ExternalInput")
    wpt = nc.dram_tensor("wpt", [E, E], BF16, kind="ExternalInput")
    bp = nc.dram_tensor("bp", [1, E], BF16, kind="ExternalInput")
    out = nc.dram_tensor("out", [SQ, E], F32, kind="ExternalOutput")

    with tile.TileContext(nc) as tc:
        _body(nc, tc, xt, wqkv, bqkv, wpt, bp, out)

    nc.compile()
    return nc


def _body(nc, tc, xt, wqkv, bqkv, wpt, bp, out):
    import contextlib
    ctx = contextlib.ExitStack()
    with ctx:
        constp = ctx.enter_context(tc.tile_pool(name="constp", bufs=1))
        wqp = ctx.enter_context(tc.tile_pool(name="wqp", bufs=1))
        xtp = ctx.enter_context(tc.tile_pool(name="xtp", bufs=1))
        qkvp = ctx.enter_context(tc.tile_pool(name="qkvp", bufs=1))
        vop = ctx.enter_context(tc.tile_pool(name="vop", bufs=1))
        wptp = ctx.enter_context(tc.tile_pool(name="wptp", bufs=1))
        atp = ctx.enter_context(tc.tile_pool(name="atp", bufs=1))
        expp = ctx.enter_context(tc.tile_pool(name="expp", bufs=4))
        stagep = ctx.enter_context(tc.tile_pool(name="stagep", bufs=2))
        smallp = ctx.enter_context(tc.tile_pool(name="smallp", bufs=4))
        outp = ctx.enter_context(tc.tile_pool(name="outp", bufs=2))
        psmm = ctx.enter_context(tc.tile_pool(name="psmm", bufs=2, space="PSUM"))
        pslog = ctx.enter_context(tc.tile_pool(name="pslog", bufs=2, space="PSUM"))
        psav = ctx.enter_context(tc.tile_pool(name="psav", bufs=1, space="PSUM"))
        dramp = ctx.enter_context(tc.tile_pool(name="dramp", bufs=1, space="DRAM"))

        # ---- constants (built in f32, cast-copied to bf16) --------------
        ident_f = constp.tile([P, P], F32, name="ident_f")
        make_identity(nc, ident_f[:, :])
        ident = constp.tile([P, P], BF16, name="ident")
        nc.vector.tensor_copy(ident[:, :], ident_f[:, :])
        tri_f = constp.tile([P, P], F32, name="tri_f")  # tri[k,q] = 1 if q >= k
        make_upper_triangular(nc, tri_f[:, :], val=1.0, diag=True)
        tri = constp.tile([P, P], BF16, name="tri")
        nc.vector.tensor_copy(tri[:, :], tri_f[:, :])
        ones_f = constp.tile([P, 1], F32, name="ones_f")
        nc.vector.memset(ones_f[:, :], 1.0)
        ones_r = constp.tile([1, D], F32, name="ones_r")
        nc.vector.memset(ones_r[:, :], 1.0)
        ones1 = constp.tile([1, P], BF16, name="ones1")
        nc.vector.tensor_copy(ones1[:, :], ones_f[0:1, 0:1].to_broadcast((1, P)))

        bq_sb = constp.tile([P, 3], F32, name="bq_sb")
        nc.sync.dma_start(bq_sb[:, :], bqkv[:, :])

        # dependency-free warm-up matmuls: keep the PE busy through the HAM
        # activity window while the input DMAs are in flight, so the real
        # matmuls start at 2.4 GHz instead of 1.2
        warm = constp.tile([P, QC], BF16, name="warm")
        nc.vector.memset(warm[:, :], 0.0)
        for _ in range(18):
            wp_ps = psmm.tile([P, QC], F32, tag="mmp", name="warm_ps")
            nc.tensor.matmul(wp_ps[:, :], lhsT=warm[:, 0:P], rhs=warm[:, :],
                             start=True, stop=True)

        # ---- weight + activation loads (per contraction chunk) ----------
        wq_sb = [wqp.tile([P, 3 * F], BF16, name=f"wq_sb{k}", tag=f"wq{k}")
                 for k in range(KCH)]
        xt_sb = [xtp.tile([P, S], BF16, name=f"xt_sb{k}", tag=f"xt{k}")
                 for k in range(KCH)]
        # split across the two HWDGE queues (sync + scalar)
        for k in range(KCH):
            eng = nc.sync if k % 2 == 0 else nc.scalar
            eng.dma_start(wq_sb[k][:, :], wqkv[k * P:(k + 1) * P, :])
            eng.dma_start(xt_sb[k][:, :], xt[k * P:(k + 1) * P, :])

        qkv_sb = [qkvp.tile([P, 3, QC], BF16, name=f"qkv_sb{n}", tag=f"qkv{n}")
                  for n in range(NQC)]
        vones = [vop.tile([P, 2 * (D + 1)], BF16, name=f"vones{kb}",
                          tag=f"vo{kb}") for kb in range(NKB)]

        # a2a groups (out row order stays qc 1,2,3,0)
        GROUPS = [[1, 2], [3, 0]]
        NG = len(GROUPS)
        a2a_in = [dramp.tile([NCORES * F, len(GROUPS[g]) * ST], BF16,
                             name=f"a2a_in{g}", tag=f"ai{g}")
                  for g in range(NG)]
        a2a_out = [dramp.tile([NCORES * F, len(GROUPS[g]) * ST], BF16,
                              name=f"a2a_out{g}", tag=f"ao{g}")
                   for g in range(NG)]
        at_sb = [atp.tile([P, KCH, len(GROUPS[g]) * ST], BF16,
                          name=f"at_sb{g}", tag=f"at{g}") for g in range(NG)]
        wp_sb = wptp.tile([P, KCH, E], BF16, name="wp_sb")
        bp_sb = constp.tile([1, E], BF16, name="bp_sb")

        def emit_qkv_m(n, m):
            nsl = slice(n * QC, (n + 1) * QC)
            pt = psmm.tile([P, QC], F32, tag="mmp", name="qkv_ps")
            for k in range(KCH):
                nc.tensor.matmul(
                    pt[:, :], lhsT=wq_sb[k][:, m * P:(m + 1) * P],
                    rhs=xt_sb[k][:, nsl],
                    start=(k == 0), stop=(k == KCH - 1))
            nc.vector.tensor_add(
                qkv_sb[n][:, m, :], pt[:, :],
                bq_sb[:, m:m + 1].to_broadcast((P, QC)))

        def emit_qkv(n):
            for m in range(3):
                emit_qkv_m(n, m)

        def emit_vtrans(kb):
            n = kb // 4
            tp = psmm.tile([P, QC], BF16, tag="mmp", name="vt_ps")
            nc.tensor.transpose(
                tp[:, :P], qkv_sb[n][:, 2, (kb % 4) * P:(kb % 4 + 1) * P],
                ident[:, :])
            vo = vones[kb]
            nc.vector.tensor_copy(vo[:, 0:D], tp[:, 0:D])
            nc.vector.tensor_copy(vo[:, D + 1:2 * D + 1], tp[:, D:2 * D])
            nc.vector.tensor_copy(vo[:, D:D + 1], ones_f[:, :])
            nc.vector.tensor_copy(vo[:, 2 * D + 1:2 * D + 2], ones_f[:, :])

        def emit_attn(qc, fillers=()):
            # fillers: emission callables sprinkled between k blocks so the
            # PE keeps independent work queued while exp stalls attention
            fillers = list(fillers)
            nkb = 4 * qc + 4
            avp = [psav.tile([D + 1, QC], F32, tag=f"avp{h}",
                             name=f"av_ps{h}") for h in range(HPC)]
            pend = []  # deferred attn@v (2 k blocks deep)

            def flush(item, last):
                kb, et, qoff, N = item
                mm = None
                for h in range(HPC):
                    mm = nc.tensor.matmul(
                        avp[h][:, qoff:QC],
                        lhsT=vones[kb][:, h * (D + 1):(h + 1) * (D + 1)],
                        rhs=et[:, h, :N],
                        start=(kb == 0), stop=last)
                return mm

            for kb in range(nkb):
                diag = kb >= 4 * qc
                qoff = P * (kb - 4 * qc) if diag else 0
                N = QC - qoff
                qsl = slice(qc * QC + qoff, (qc + 1) * QC)
                lqsl = slice(qoff, QC)
                # two heads' logits into the two banks of one psum tile
                lp = pslog.tile([P, 2 * QC], F32, tag="logp", name="log_ps")
                for h in range(HPC):
                    nc.tensor.matmul(
                        lp[:, h * QC:h * QC + N],
                        lhsT=qkv_sb[kb // 4][D * h:D * (h + 1), 1,
                                             (kb % 4) * P:(kb % 4 + 1) * P],
                        rhs=qkv_sb[qc][D * h:D * (h + 1), 0, lqsl],
                        start=True, stop=True)
                et = expp.tile([P, 2, QC], BF16, tag="et", name="exp_sb")
                nc.scalar.activation(
                    et[:, :, :N],
                    lp[:, :].rearrange("p (b n) -> p b n", b=2)[:, :, :N],
                    EXP)
                if diag:
                    nc.vector.tensor_mul(
                        et[:, :, 0:P], et[:, :, 0:P],
                        tri[:, None, :].to_broadcast((P, 2, P)))
                if len(pend) >= 2:
                    flush(pend.pop(0), False)
                pend.append((kb, et, qoff, N))
                if fillers and kb % 2 == 1:
                    fillers.pop(0)()
            for f in fillers:
                f()
            last_av = None
            while pend:
                last_av = flush(pend.pop(0), not pend)

            # normalize rows 0:64 by row 64 (exp sums), both heads into one
            # staging tile, then scatter stripes into the a2a input buffer
            g, slot = next((g, s) for g in range(NG)
                           for s in range(len(GROUPS[g])) if GROUPS[g][s] == qc)
            stage = stagep.tile([P, QC], BF16, tag="stage", name="stage")
            for h in range(HPC):
                rs = smallp.tile([1, QC], F32, tag="rs", name="rs")
                nc.vector.tensor_copy(rs[:, :], avp[h][D:D + 1, :])
                rr = smallp.tile([1, QC], F32, tag="rr", name="rr")
                nc.vector.reciprocal_approx_fast(rr[:, :], rs[:, :])
                rb = smallp.tile([D, QC], F32, tag="rb", name="rb")
                nc.gpsimd.partition_broadcast(rb[:, :], rr[:, :])
                nc.vector.tensor_mul(
                    stage[D * h:D * (h + 1), :], avp[h][0:D, :], rb[:, :])
            nc.sync.dma_start(
                a2a_in[g][:, :].rearrange("(j r) q -> r j q", r=P)
                [:, :, slot * ST:(slot + 1) * ST],
                stage[:, :].rearrange("p (j q) -> p j q", q=ST))
            if slot == len(GROUPS[g]) - 1:
                nc.gpsimd.collective_compute(
                    "AllToAll", mybir.AluOpType.bypass,
                    replica_groups=[list(range(NCORES))],
                    ins=[a2a_in[g][:, :].opt()],
                    outs=[a2a_out[g][:, :].opt()])
            return last_av

        def emit_proj(ph, anchor):
            # projection for stripe pair (2*ph, 2*ph+1): 128 output rows.
            # anchor: keep these instructions behind the attention stream in
            # the static schedule - the scheduler otherwise hoists them and
            # the collective-gated loads head-of-line block the PE/DMA FIFOs.
            def pin(inst):
                if anchor is not None:
                    add_dep_helper(inst.ins, anchor.ins, sync=False,
                                   reason="proj after attention")
                return inst
            # stripe s (0..3, out-row order) lives in group g at slot
            def stripe_src(s, k):
                acc = 0
                for g in range(NG):
                    if s < acc + len(GROUPS[g]):
                        slot = s - acc
                        return at_sb[g][:, k, slot * ST:(slot + 1) * ST]
                    acc += len(GROUPS[g])
            ob = outp.tile([P, E], F32, tag="ob", name="ob")
            for n in range(E // QC):
                nsl = slice(n * QC, (n + 1) * QC)
                pp = psmm.tile([P, QC], F32, tag="mmp", name="proj_ps")
                for k in range(KCH):
                    for half in range(2):
                        pin(nc.tensor.matmul(
                            pp[half * ST:(half + 1) * ST, :],
                            lhsT=stripe_src(2 * ph + half, k),
                            rhs=wp_sb[:, k, nsl],
                            start=(k == 0), stop=False,
                            tile_position=(0, half * ST)))
                pin(nc.tensor.matmul(
                    pp[:, :], lhsT=ones1[:, :], rhs=bp_sb[:, nsl],
                    start=False, stop=True))
                nc.vector.tensor_copy(ob[:, nsl], pp[:, :])
            nc.sync.dma_start(out[ph * P:(ph + 1) * P, :], ob[:, :])

        # ---- emission: interleave qkv chunks with attention so both PE
        # phases and the ACT exp stream overlap; attention runs 1,2,3,0 so
        # the final collective only waits on the shortest chunk (qc0).
        emit_qkv(0)
        for kb in range(0, 4):
            emit_vtrans(kb)
        emit_qkv(1)
        for kb in range(4, 8):
            emit_vtrans(kb)
        for k in range(KCH):
            eng = nc.sync if k % 2 == 0 else nc.scalar
            eng.dma_start(wp_sb[:, k, :], wpt[k * P:(k + 1) * P, :])
        nc.sync.dma_start(bp_sb[:, :], bp[:, :])
        f1 = [lambda m=m: emit_qkv_m(2, m) for m in range(3)] + \
             [lambda kb=kb: emit_vtrans(kb) for kb in range(8, 12)]
        emit_attn(1, f1)
        f2 = [lambda m=m: emit_qkv_m(3, m) for m in range(3)] + \
             [lambda kb=kb: emit_vtrans(kb) for kb in range(12, 16)]
        emit_attn(2, f2)
        emit_attn(3)
        last_av = emit_attn(0)
        for g in range(NG):
            d = nc.sync.dma_start(
                at_sb[g][:, :, :],
                a2a_out[g][:, :].rearrange("(kc p) q -> p kc q", p=P))
            add_dep_helper(d.ins, last_av.ins, sync=False,
                           reason="gathered loads after attention")
        for ph in range(2):
            emit_proj(ph, last_av)


_NC_CACHE = None


def _get_nc():
    global _NC_CACHE
    if _NC_CACHE is None:
        _NC_CACHE = build_nc()
    return _NC_CACHE


def make_in_maps(x, w_attn, b_attn, w_proj, b_proj):
    bf16 = ml_dtypes.bfloat16
    x = np.asarray(x, dtype=np.float32)
    w_attn = np.asarray(w_attn, dtype=np.float32)
    b_attn = np.asarray(b_attn, dtype=np.float32)
    w_proj = np.asarray(w_proj, dtype=np.float32)
    b_proj = np.asarray(b_proj, dtype=np.float32)

    xt = np.ascontiguousarray(x.T).astype(bf16)          # (E, S)
    wpt = np.ascontiguousarray(w_proj.T).astype(bf16)    # (E, E)
    bpa = np.ascontiguousarray(b_proj[None, :]).astype(bf16)
    scale = 1.0 / np.sqrt(D)

    in_maps = []
    for c in range(NCORES):
        rq = slice(F * c, F * (c + 1))
        rk = slice(E + F * c, E + F * (c + 1))
        rv = slice(2 * E + F * c, 2 * E + F * (c + 1))
        wqkv = np.ascontiguousarray(np.concatenate(
            [w_attn[rq] * scale, w_attn[rk], w_attn[rv]], axis=0).T)
        bq = np.stack([b_attn[rq] * scale, b_attn[rk], b_attn[rv]], axis=1)
        in_maps.append({
            "xt": xt,
            "wqkv": wqkv.astype(bf16),
            "bqkv": np.ascontiguousarray(bq, dtype=np.float32),
            "wpt": wpt,
            "bp": bpa,
        })
    return in_maps


def run(inputs, trace=False, **kw):
    from concourse.bass_utils import run_bass_kernel_spmd
    nc = _get_nc()
    in_maps = make_in_maps(**inputs)
    res = run_bass_kernel_spmd(nc, in_maps, core_ids=list(range(NCORES)),
                               trace=trace, **kw)
    # core j's out row blocks are stripes for qc = 3,2,1,0 in that order;
    # stripe qc covers global rows 512*qc + 64*j .. +64
    full = np.empty((S, E), dtype=np.float32)
    for j in range(NCORES):
        o = res.results[j]["out"]                        # (256, E)
        for blk, qc in enumerate([1, 2, 3, 0]):
            full[QC * qc + ST * j: QC * qc + ST * (j + 1), :] = \
                o[ST * blk: ST * (blk + 1), :]
    return full, res


def kernel(**inputs):
    full, _ = run(inputs, trace=False)
    return full


# revision 37
# speedup vs baseline: 1.1369x; 1.0066x over previous
"""Distributed causal self-attention for TRN2 (8 NeuronCores).

Sharding: tensor-parallel over heads (2 heads/core). Each core computes
q,k,v for its heads over the full sequence (column-sharded c_attn), runs
causal attention for them (chunk order 1,2,3,0 so the last chunk is the
shortest), reshards the attention output sequence-wise with 2 chunked
AllToAlls (the first overlapped with later attention chunks), and applies
the full output projection to its 256 rows (row-sharded c_proj).

Row ownership is striped: within q-chunk qc (512 rows), rows
[512*qc + 64*j : 512*qc + 64*(j+1)] belong to core j. Core j's "out"
holds its 4 stripes in qc order; the host reassembles.

Compute dtype: bf16 operands, fp32 PSUM accumulation.

Per-core layouts (S=2048, E=1024, D=64, F=128 local feats):
  xt   (E, S)  bf16  x^T               wqkv (E, 3F) bf16  [Wq*s|Wk|Wv]^T
  bqkv (128,3) f32   bias columns      wpt  (E, E)  bf16  w_proj^T
  bp   (1, E)  bf16  b_proj            out  (256,E) f32
"""

import numpy as np
import ml_dtypes

import concourse.bass as bass
import concourse.mybir as mybir
import concourse.tile as tile
from concourse import bacc
from concourse.masks import make_identity, make_upper_triangular
from concourse.tile import add_dep_helper

S, E, H = 2048, 1024, 16
D = E // H          # 64 head dim
NCORES = 8
HPC = H // NCORES   # 2 heads per core
F = HPC * D         # 128 local features
SQ = S // NCORES    # 256 output rows per core
ST = 64             # per-core stripe within a q chunk
P = 128
QC = 512            # q chunk (columns per attention pass)
NQC = S // QC       # 4
NKB = S // P        # 16 k blocks
KCH = E // P        # 8 contraction chunks for E-dim matmuls

F32 = mybir.dt.float32
BF16 = mybir.dt.bfloat16
EXP = mybir.ActivationFunctionType.Exp


def build_nc():
    nc = bacc.Bacc("TRN2", target_bir_lowering=False, debug=False,
                   num_devices=NCORES, enable_partition_id=True)

    xt = nc.dram_tensor("xt", [E, S], BF16, kind="ExternalInput")
    wqkv = nc.dram_tensor("wqkv", [E, 3 * F], BF16, kind="ExternalInput")
    bqkv = nc.dram_tensor("bqkv", [P, 3], F32, kind="ExternalInput")
    wpt = nc.dram_tensor("wpt", [E, E], BF16, kind="ExternalInput")
    bp = nc.dram_tensor("bp", [1, E], BF16, kind="ExternalInput")
    out = nc.dram_tensor("out", [SQ, E], F32, kind="ExternalOutput")

    with tile.TileContext(nc) as tc:
        _body(nc, tc, xt, wqkv, bqkv, wpt, bp, out)

    nc.compile()
    return nc


def _body(nc, tc, xt, wqkv, bqkv, wpt, bp, out):
    import contextlib
    ctx = contextlib.ExitStack()
    with ctx:
        constp = ctx.enter_context(tc.tile_pool(name="constp", bufs=1))
        wqp = ctx.enter_context(tc.tile_pool(name="wqp", bufs=1))
        xtp = ctx.enter_context(tc.tile_pool(name="xtp", bufs=1))
        qkvp = ctx.enter_context(tc.tile_pool(name="qkvp", bufs=1))
        vop = ctx.enter_context(tc.tile_pool(name="vop", bufs=1))
        wptp = ctx.enter_context(tc.tile_pool(name="wptp", bufs=1))
        atp = ctx.enter_context(tc.tile_pool(name="atp", bufs=1))
        expp = ctx.enter_context(tc.tile_pool(name="expp", bufs=4))
        stagep = ctx.enter_context(tc.tile_pool(name="stagep", bufs=2))
        smallp = ctx.enter_context(tc.tile_pool(name="smallp", bufs=4))
        outp = ctx.enter_context(tc.tile_pool(name="outp", bufs=2))
        psmm = ctx.enter_context(tc.tile_pool(name="psmm", bufs=2, space="PSUM"))
        pslog = ctx.enter_context(tc.tile_pool(name="pslog", bufs=2, space="PSUM"))
        psav = ctx.enter_context(tc.tile_pool(name="psav", bufs=1, space="PSUM"))
        dramp = ctx.enter_context(tc.tile_pool(name="dramp", bufs=1, space="DRAM"))

        # ---- constants (built in f32, cast-copied to bf16) --------------
        ident_f = constp.tile([P, P], F32, name="ident_f")
        make_identity(nc, ident_f[:, :])
        ident = constp.tile([P, P], BF16, name="ident")
        nc.vector.tensor_copy(ident[:, :], ident_f[:, :])
        tri_f = constp.tile([P, P], F32, name="tri_f")  # tri[k,q] = 1 if q >= k
        make_upper_triangular(nc, tri_f[:, :], val=1.0, diag=True)
        tri = constp.tile([P, P], BF16, name="tri")
        nc.vector.tensor_copy(tri[:, :], tri_f[:, :])
        ones_f = constp.tile([P, 1], F32, name="ones_f")
        nc.vector.memset(ones_f[:, :], 1.0)
        ones_r = constp.tile([1, D], F32, name="ones_r")
        nc.vector.memset(ones_r[:, :], 1.0)
        ones1 = constp.tile([1, P], BF16, name="ones1")
        nc.vector.tensor_copy(ones1[:, :], ones_f[0:1, 0:1].to_broadcast((1, P)))

        bq_sb = constp.tile([P, 3], F32, name="bq_sb")
        nc.sync.dma_start(bq_sb[:, :], bqkv[:, :])

        # dependency-free warm-up matmuls: keep the PE busy through the HAM
        # activity window while the input DMAs are in flight, so the real
        # matmuls start at 2.4 GHz instead of 1.2
        warm = constp.tile([P, QC], BF16, name="warm")
        nc.vector.memset(warm[:, :], 0.0)
        for _ in range(18):
            wp_ps = psmm.tile([P, QC], F32, tag="mmp", name="warm_ps")
            nc.tensor.matmul(wp_ps[:, :], lhsT=warm[:, 0:P], rhs=warm[:, :],
                             start=True, stop=True)

        # ---- weight + activation loads (per contraction chunk) ----------
        wq_sb = [wqp.tile([P, 3 * F], BF16, name=f"wq_sb{k}", tag=f"wq{k}")
                 for k in range(KCH)]
        xt_sb = [xtp.tile([P, S], BF16, name=f"xt_sb{k}", tag=f"xt{k}")
                 for k in range(KCH)]
        # split across the two HWDGE queues (sync + scalar)
        for k in range(KCH):
            eng = nc.sync if k % 2 == 0 else nc.scalar
            eng.dma_start(wq_sb[k][:, :], wqkv[k * P:(k + 1) * P, :])
            eng.dma_start(xt_sb[k][:, :], xt[k * P:(k + 1) * P, :])

        # separate q/k/v tiles per chunk so consumers only wait on the
        # piece they read (whole-tile deps otherwise delay attention start)
        qkv_sb = [[qkvp.tile([P, QC], BF16, name=f"qkv_sb{n}_{m}",
                             tag=f"qkv{n}_{m}") for m in range(3)]
                  for n in range(NQC)]
        vones = [vop.tile([P, 2 * (D + 1)], BF16, name=f"vones{kb}",
                          tag=f"vo{kb}") for kb in range(NKB)]

        # a2a groups (out row order stays qc 1,2,3,0)
        GROUPS = [[1, 2], [3, 0]]
        NG = len(GROUPS)
        a2a_in = [dramp.tile([NCORES * F, len(GROUPS[g]) * ST], BF16,
                             name=f"a2a_in{g}", tag=f"ai{g}")
                  for g in range(NG)]
        a2a_out = [dramp.tile([NCORES * F, len(GROUPS[g]) * ST], BF16,
                              name=f"a2a_out{g}", tag=f"ao{g}")
                   for g in range(NG)]
        at_sb = [atp.tile([P, KCH, len(GROUPS[g]) * ST], BF16,
                          name=f"at_sb{g}", tag=f"at{g}") for g in range(NG)]
        wp_sb = wptp.tile([P, KCH, E], BF16, name="wp_sb")
        bp_sb = constp.tile([1, E], BF16, name="bp_sb")

        def emit_qkv_m(n, m):
            nsl = slice(n * QC, (n + 1) * QC)
            pt = psmm.tile([P, QC], F32, tag="mmp", name="qkv_ps")
            for k in range(KCH):
                nc.tensor.matmul(
                    pt[:, :], lhsT=wq_sb[k][:, m * P:(m + 1) * P],
                    rhs=xt_sb[k][:, nsl],
                    start=(k == 0), stop=(k == KCH - 1))
            nc.vector.tensor_add(
                qkv_sb[n][m][:, :], pt[:, :],
                bq_sb[:, m:m + 1].to_broadcast((P, QC)))

        def emit_qkv(n):
            for m in range(3):
                emit_qkv_m(n, m)

        def emit_vtrans(kb):
            n = kb // 4
            tp = psmm.tile([P, QC], BF16, tag="mmp", name="vt_ps")
            nc.tensor.transpose(
                tp[:, :P], qkv_sb[n][2][:, (kb % 4) * P:(kb % 4 + 1) * P],
                ident[:, :])
            vo = vones[kb]
            nc.vector.tensor_copy(vo[:, 0:D], tp[:, 0:D])
            nc.vector.tensor_copy(vo[:, D + 1:2 * D + 1], tp[:, D:2 * D])
            nc.vector.tensor_copy(vo[:, D:D + 1], ones_f[:, :])
            nc.vector.tensor_copy(vo[:, 2 * D + 1:2 * D + 2], ones_f[:, :])

        def emit_attn(qc, fillers=()):
            # fillers: emission callables sprinkled between k blocks so the
            # PE keeps independent work queued while exp stalls attention
            fillers = list(fillers)
            nkb = 4 * qc + 4
            avp = [psav.tile([D + 1, QC], F32, tag=f"avp{h}",
                             name=f"av_ps{h}") for h in range(HPC)]
            pend = []  # deferred attn@v (2 k blocks deep)

            def flush(item, last):
                kb, et, qoff, N = item
                mm = None
                for h in range(HPC):
                    mm = nc.tensor.matmul(
                        avp[h][:, qoff:QC],
                        lhsT=vones[kb][:, h * (D + 1):(h + 1) * (D + 1)],
                        rhs=et[:, h, :N],
                        start=(kb == 0), stop=last)
                return mm

            for kb in range(nkb):
                diag = kb >= 4 * qc
                qoff = P * (kb - 4 * qc) if diag else 0
                N = QC - qoff
                qsl = slice(qc * QC + qoff, (qc + 1) * QC)
                lqsl = slice(qoff, QC)
                # two heads' logits into the two banks of one psum tile
                lp = pslog.tile([P, 2 * QC], F32, tag="logp", name="log_ps")
                for h in range(HPC):
                    nc.tensor.matmul(
                        lp[:, h * QC:h * QC + N],
                        lhsT=qkv_sb[kb // 4][1][D * h:D * (h + 1),
                                                (kb % 4) * P:(kb % 4 + 1) * P],
                        rhs=qkv_sb[qc][0][D * h:D * (h + 1), lqsl],
                        start=True, stop=True)
                et = expp.tile([P, 2, QC], BF16, tag="et", name="exp_sb")
                nc.scalar.activation(
                    et[:, :, :N],
                    lp[:, :].rearrange("p (b n) -> p b n", b=2)[:, :, :N],
                    EXP)
                if diag:
                    nc.vector.tensor_mul(
                        et[:, :, 0:P], et[:, :, 0:P],
                        tri[:, None, :].to_broadcast((P, 2, P)))
                if len(pend) >= 2:
                    flush(pend.pop(0), False)
                pend.append((kb, et, qoff, N))
                if fillers and kb % 2 == 1:
                    fillers.pop(0)()
            for f in fillers:
                f()
            last_av = None
            while pend:
                last_av = flush(pend.pop(0), not pend)

            # normalize rows 0:64 by row 64 (exp sums), both heads into one
            # staging tile, then scatter stripes into the a2a input buffer
            g, slot = next((g, s) for g in range(NG)
                           for s in range(len(GROUPS[g])) if GROUPS[g][s] == qc)
            stage = stagep.tile([P, QC], BF16, tag="stage", name="stage")
            for h in range(HPC):
                rs = smallp.tile([1, QC], F32, tag="rs", name="rs")
                nc.vector.tensor_copy(rs[:, :], avp[h][D:D + 1, :])
                rr = smallp.tile([1, QC], F32, tag="rr", name="rr")
                nc.vector.reciprocal_approx_fast(rr[:, :], rs[:, :])
                rb = smallp.tile([D, QC], F32, tag="rb", name="rb")
                nc.gpsimd.partition_broadcast(rb[:, :], rr[:, :])
                nc.vector.tensor_mul(
                    stage[D * h:D * (h + 1), :], avp[h][0:D, :], rb[:, :])
            for h in range(HPC):
                nc.sync.dma_start(
                    a2a_in[g][:, :].rearrange("(j r) q -> r j q", r=P)
                    [D * h:D * (h + 1), :, slot * ST:(slot + 1) * ST],
                    stage[D * h:D * (h + 1), :]
                    .rearrange("p (j q) -> p j q", q=ST))
            if slot == len(GROUPS[g]) - 1:
                nc.gpsimd.collective_compute(
                    "AllToAll", mybir.AluOpType.bypass,
                    replica_groups=[list(range(NCORES))],
                    ins=[a2a_in[g][:, :].opt()],
                    outs=[a2a_out[g][:, :].opt()])
            return last_av

        def emit_proj(ph, anchor):
            # projection for stripe pair (2*ph, 2*ph+1): 128 output rows.
            # anchor: keep these instructions behind the attention stream in
            # the static schedule - the scheduler otherwise hoists them and
            # the collective-gated loads head-of-line block the PE/DMA FIFOs.
            def pin(inst):
                if anchor is not None:
                    add_dep_helper(inst.ins, anchor.ins, sync=False,
                                   reason="proj after attention")
                return inst
            # stripe s (0..3, out-row order) lives in group g at slot
            def stripe_src(s, k):
                acc = 0
                for g in range(NG):
                    if s < acc + len(GROUPS[g]):
                        slot = s - acc
                        return at_sb[g][:, k, slot * ST:(slot + 1) * ST]
                    acc += len(GROUPS[g])
            ob = outp.tile([P, E], F32, tag="ob", name="ob")
            for n in range(E // QC):
                nsl = slice(n * QC, (n + 1) * QC)
                pp = psmm.tile([P, QC], F32, tag="mmp", name="proj_ps")
                for k in range(KCH):
                    for half in range(2):
                        pin(nc.tensor.matmul(
                            pp[half * ST:(half + 1) * ST, :],
                            lhsT=stripe_src(2 * ph + half, k),
                            rhs=wp_sb[:, k, nsl],
                            start=(k == 0), stop=False,
                            tile_position=(0, half * ST)))
                pin(nc.tensor.matmul(
                    pp[:, :], lhsT=ones1[:, :], rhs=bp_sb[:, nsl],
                    start=False, stop=True))
                nc.vector.tensor_copy(ob[:, nsl], pp[:, :])
            nc.sync.dma_start(out[ph * P:(ph + 1) * P, :], ob[:, :])

        # ---- emission: interleave qkv chunks with attention so both PE
        # phases and the ACT exp stream overlap; attention runs 1,2,3,0 so
        # the final collective only waits on the shortest chunk (qc0).
        emit_qkv(0)
        for kb in range(0, 4):
            emit_vtrans(kb)
        emit_qkv(1)
        for kb in range(4, 8):
            emit_vtrans(kb)
        for k in range(KCH):
            eng = nc.sync if k % 2 == 0 else nc.scalar
            eng.dma_start(wp_sb[:, k, :], wpt[k * P:(k + 1) * P, :])
        nc.sync.dma_start(bp_sb[:, :], bp[:, :])
        f1 = [lambda m=m: emit_qkv_m(2, m) for m in range(3)] + \
             [lambda kb=kb: emit_vtrans(kb) for kb in range(8, 12)]
        emit_attn(1, f1)
        f2 = [lambda m=m: emit_qkv_m(3, m) for m in range(3)] + \
             [lambda kb=kb: emit_vtrans(kb) for kb in range(12, 16)]
        emit_attn(2, f2)
        emit_attn(3)
        last_av = emit_attn(0)
        for g in range(NG):
            d = nc.sync.dma_start(
                at_sb[g][:, :, :],
                a2a_out[g][:, :].rearrange("(kc p) q -> p kc q", p=P))
            add_dep_helper(d.ins, last_av.ins, sync=False,
                           reason="gathered loads after attention")
        for ph in range(2):
            emit_proj(ph, last_av)


_NC_CACHE = None


def _get_nc():
    global _NC_CACHE
    if _NC_CACHE is None:
        _NC_CACHE = build_nc()
    return _NC_CACHE


def make_in_maps(x, w_attn, b_attn, w_proj, b_proj):
    bf16 = ml_dtypes.bfloat16
    x = np.asarray(x, dtype=np.float32)
    w_attn = np.asarray(w_attn, dtype=np.float32)
    b_attn = np.asarray(b_attn, dtype=np.float32)
    w_proj = np.asarray(w_proj, dtype=np.float32)
    b_proj = np.asarray(b_proj, dtype=np.float32)

    xt = np.ascontiguousarray(x.T).astype(bf16)          # (E, S)
    wpt = np.ascontiguousarray(w_proj.T).astype(bf16)    # (E, E)
    bpa = np.ascontiguousarray(b_proj[None, :]).astype(bf16)
    scale = 1.0 / np.sqrt(D)

    in_maps = []
    for c in range(NCORES):
        rq = slice(F * c, F * (c + 1))
        rk = slice(E + F * c, E + F * (c + 1))
        rv = slice(2 * E + F * c, 2 * E + F * (c + 1))
        wqkv = np.ascontiguousarray(np.concatenate(
            [w_attn[rq] * scale, w_attn[rk], w_attn[rv]], axis=0).T)
        bq = np.stack([b_attn[rq] * scale, b_attn[rk], b_attn[rv]], axis=1)
        in_maps.append({
            "xt": xt,
            "wqkv": wqkv.astype(bf16),
            "bqkv": np.ascontiguousarray(bq, dtype=np.float32),
            "wpt": wpt,
            "bp": bpa,
        })
    return in_maps


def run(inputs, trace=False, **kw):
    from concourse.bass_utils import run_bass_kernel_spmd
    nc = _get_nc()
    in_maps = make_in_maps(**inputs)
    res = run_bass_kernel_spmd(nc, in_maps, core_ids=list(range(NCORES)),
                               trace=trace, **kw)
    # core j's out row blocks are stripes for qc = 3,2,1,0 in that order;
    # stripe qc covers global rows 512*qc + 64*j .. +64
    full = np.empty((S, E), dtype=np.float32)
    for j in range(NCORES):
        o = res.results[j]["out"]                        # (256, E)
        for blk, qc in enumerate([1, 2, 3, 0]):
            full[QC * qc + ST * j: QC * qc + ST * (j + 1), :] = \
                o[ST * blk: ST * (blk + 1), :]
    return full, res


def kernel(**inputs):
    full, _ = run(inputs, trace=False)
    return full


# revision 41
# speedup vs baseline: 1.2359x; 1.0871x over previous
"""Distributed causal self-attention for TRN2 (8 NeuronCores).

Sharding: tensor-parallel over heads (2 heads/core). Each core computes
q,k,v for its heads over the full sequence (column-sharded c_attn), runs
causal attention for them (chunk order 1,2,3,0 so the last chunk is the
shortest), reshards the attention output sequence-wise with 2 chunked
AllToAlls (the first overlapped with later attention chunks), and applies
the full output projection to its 256 rows (row-sharded c_proj).

Row ownership is striped: within q-chunk qc (512 rows), rows
[512*qc + 64*j : 512*qc + 64*(j+1)] belong to core j. Core j's "out"
holds its 4 stripes in qc order; the host reassembles.

Compute dtype: bf16 operands, fp32 PSUM accumulation.

Per-core layouts (S=2048, E=1024, D=64, F=128 local feats):
  xt   (E, S)  bf16  x^T               wqkv (E, 3F) bf16  [Wq*s|Wk|Wv]^T
  bqkv (128,3) f32   bias columns      wpt  (E, E)  bf16  w_proj^T
  bp   (1, E)  bf16  b_proj            out  (256,E) f32
"""

import numpy as np
import ml_dtypes

import concourse.bass as bass
import concourse.mybir as mybir
import concourse.tile as tile
from concourse import bacc
from concourse.masks import make_identity, make_upper_triangular
from concourse.tile import add_dep_helper

S, E, H = 2048, 1024, 16
D = E // H          # 64 head dim
NCORES = 8
HPC = H // NCORES   # 2 heads per core
F = HPC * D         # 128 local features
SQ = S // NCORES    # 256 output rows per core
ST = 64             # per-core stripe within a q chunk
P = 128
QC = 512            # q chunk (columns per attention pass)
NQC = S // QC       # 4
NKB = S // P        # 16 k blocks
KCH = E // P        # 8 contraction chunks for E-dim matmuls

F32 = mybir.dt.float32
BF16 = mybir.dt.bfloat16
EXP = mybir.ActivationFunctionType.Exp


def build_nc():
    nc = bacc.Bacc("TRN2", target_bir_lowering=False, debug=False,
                   num_devices=NCORES, enable_partition_id=True)

    xt = nc.dram_tensor("xt", [E, S], BF16, kind="ExternalInput")
    wqkv = nc.dram_tensor("wqkv", [E, 3 * F], BF16, kind="ExternalInput")
    bqkv = nc.dram_tensor("bqkv", [P, 3], F32, kind="ExternalInput")
    wpt = nc.dram_tensor("wpt", [E, E], BF16, kind="ExternalInput")
    bp = nc.dram_tensor("bp", [1, E], BF16, kind="ExternalInput")
    out = nc.dram_tensor("out", [SQ, E], F32, kind="ExternalOutput")

    with tile.TileContext(nc) as tc:
        _body(nc, tc, xt, wqkv, bqkv, wpt, bp, out)

    nc.compile()
    return nc


def _body(nc, tc, xt, wqkv, bqkv, wpt, bp, out):
    import contextlib
    ctx = contextlib.ExitStack()
    with ctx:
        constp = ctx.enter_context(tc.tile_pool(name="constp", bufs=1))
        wqp = ctx.enter_context(tc.tile_pool(name="wqp", bufs=1))
        xtp = ctx.enter_context(tc.tile_pool(name="xtp", bufs=1))
        qkvp = ctx.enter_context(tc.tile_pool(name="qkvp", bufs=1))
        vop = ctx.enter_context(tc.tile_pool(name="vop", bufs=1))
        wptp = ctx.enter_context(tc.tile_pool(name="wptp", bufs=1))
        atp = ctx.enter_context(tc.tile_pool(name="atp", bufs=1))
        expp = ctx.enter_context(tc.tile_pool(name="expp", bufs=4))
        stagep = ctx.enter_context(tc.tile_pool(name="stagep", bufs=3))
        smallp = ctx.enter_context(tc.tile_pool(name="smallp", bufs=4))
        outp = ctx.enter_context(tc.tile_pool(name="outp", bufs=2))
        psmm = ctx.enter_context(tc.tile_pool(name="psmm", bufs=2, space="PSUM"))
        pslog = ctx.enter_context(tc.tile_pool(name="pslog", bufs=2, space="PSUM"))
        psav = ctx.enter_context(tc.tile_pool(name="psav", bufs=1, space="PSUM"))
        dramp = ctx.enter_context(tc.tile_pool(name="dramp", bufs=1, space="DRAM"))

        # ---- constants (built in f32, cast-copied to bf16) --------------
        ident_f = constp.tile([P, P], F32, name="ident_f")
        make_identity(nc, ident_f[:, :])
        ident = constp.tile([P, P], BF16, name="ident")
        nc.vector.tensor_copy(ident[:, :], ident_f[:, :])
        tri_f = constp.tile([P, P], F32, name="tri_f")  # tri[k,q] = 1 if q >= k
        make_upper_triangular(nc, tri_f[:, :], val=1.0, diag=True)
        tri = constp.tile([P, P], BF16, name="tri")
        nc.vector.tensor_copy(tri[:, :], tri_f[:, :])
        ones_f = constp.tile([P, 1], F32, name="ones_f")
        nc.vector.memset(ones_f[:, :], 1.0)
        ones_r = constp.tile([1, D], F32, name="ones_r")
        nc.vector.memset(ones_r[:, :], 1.0)
        ones1 = constp.tile([1, P], BF16, name="ones1")
        nc.vector.tensor_copy(ones1[:, :], ones_f[0:1, 0:1].to_broadcast((1, P)))

        bq_sb = constp.tile([P, 3], F32, name="bq_sb")
        nc.sync.dma_start(bq_sb[:, :], bqkv[:, :])

        # dependency-free warm-up matmuls: keep the PE busy through the HAM
        # activity window while the input DMAs are in flight, so the real
        # matmuls start at 2.4 GHz instead of 1.2
        warm = constp.tile([P, QC], BF16, name="warm")
        nc.vector.memset(warm[:, :], 0.0)
        for _ in range(18):
            wp_ps = psmm.tile([P, QC], F32, tag="mmp", name="warm_ps")
            nc.tensor.matmul(wp_ps[:, :], lhsT=warm[:, 0:P], rhs=warm[:, :],
                             start=True, stop=True)

        # ---- weight + activation loads (per contraction chunk) ----------
        wq_sb = [wqp.tile([P, 3 * F], BF16, name=f"wq_sb{k}", tag=f"wq{k}")
                 for k in range(KCH)]
        xt_sb = [xtp.tile([P, S], BF16, name=f"xt_sb{k}", tag=f"xt{k}")
                 for k in range(KCH)]
        # split across the two HWDGE queues (sync + scalar)
        for k in range(KCH):
            eng = nc.sync if k % 2 == 0 else nc.scalar
            eng.dma_start(wq_sb[k][:, :], wqkv[k * P:(k + 1) * P, :])
            eng.dma_start(xt_sb[k][:, :], xt[k * P:(k + 1) * P, :])

        # separate q/k/v tiles per chunk so consumers only wait on the
        # piece they read (whole-tile deps otherwise delay attention start)
        qkv_sb = [[qkvp.tile([P, QC], BF16, name=f"qkv_sb{n}_{m}",
                             tag=f"qkv{n}_{m}") for m in range(3)]
                  for n in range(NQC)]
        vones = [vop.tile([P, 2 * (D + 1)], BF16, name=f"vones{kb}",
                          tag=f"vo{kb}") for kb in range(NKB)]

        # a2a groups (out row order stays qc 1,2,3,0)
        GROUPS = [[1, 2], [3, 0]]
        NG = len(GROUPS)
        a2a_in = [dramp.tile([NCORES * F, len(GROUPS[g]) * ST], BF16,
                             name=f"a2a_in{g}", tag=f"ai{g}")
                  for g in range(NG)]
        a2a_out = [dramp.tile([NCORES * F, len(GROUPS[g]) * ST], BF16,
                              name=f"a2a_out{g}", tag=f"ao{g}")
                   for g in range(NG)]
        at_sb = [atp.tile([P, KCH, len(GROUPS[g]) * ST], BF16,
                          name=f"at_sb{g}", tag=f"at{g}") for g in range(NG)]
        wp_sb = wptp.tile([P, KCH, E], BF16, name="wp_sb")
        bp_sb = constp.tile([1, E], BF16, name="bp_sb")

        def emit_qkv_m(n, m):
            nsl = slice(n * QC, (n + 1) * QC)
            pt = psmm.tile([P, QC], F32, tag="mmp", name="qkv_ps")
            for k in range(KCH):
                nc.tensor.matmul(
                    pt[:, :], lhsT=wq_sb[k][:, m * P:(m + 1) * P],
                    rhs=xt_sb[k][:, nsl],
                    start=(k == 0), stop=(k == KCH - 1))
            nc.vector.tensor_add(
                qkv_sb[n][m][:, :], pt[:, :],
                bq_sb[:, m:m + 1].to_broadcast((P, QC)))

        def emit_qkv(n):
            for m in range(3):
                emit_qkv_m(n, m)

        def emit_vtrans(kb):
            n = kb // 4
            tp = psmm.tile([P, QC], BF16, tag="mmp", name="vt_ps")
            nc.tensor.transpose(
                tp[:, :P], qkv_sb[n][2][:, (kb % 4) * P:(kb % 4 + 1) * P],
                ident[:, :])
            vo = vones[kb]
            nc.vector.tensor_copy(vo[:, 0:D], tp[:, 0:D])
            nc.vector.tensor_copy(vo[:, D + 1:2 * D + 1], tp[:, D:2 * D])
            nc.vector.tensor_copy(vo[:, D:D + 1], ones_f[:, :])
            nc.vector.tensor_copy(vo[:, 2 * D + 1:2 * D + 2], ones_f[:, :])

        def emit_attn(qc, fillers=()):
            # fillers: emission callables sprinkled between k blocks so the
            # PE keeps independent work queued while exp stalls attention
            fillers = list(fillers)
            nkb = 4 * qc + 4
            avp = [psav.tile([D + 1, QC], F32, tag=f"avp{h}",
                             name=f"av_ps{h}") for h in range(HPC)]
            pend = []  # deferred attn@v (2 k blocks deep)

            def flush(item, last):
                kb, et, qoff, N = item
                mm = None
                for h in range(HPC):
                    mm = nc.tensor.matmul(
                        avp[h][:, qoff:QC],
                        lhsT=vones[kb][:, h * (D + 1):(h + 1) * (D + 1)],
                        rhs=et[:, h, :N],
                        start=(kb == 0), stop=last)
                return mm

            for kb in range(nkb):
                diag = kb >= 4 * qc
                qoff = P * (kb - 4 * qc) if diag else 0
                N = QC - qoff
                qsl = slice(qc * QC + qoff, (qc + 1) * QC)
                lqsl = slice(qoff, QC)
                # two heads' logits into the two banks of one psum tile
                lp = pslog.tile([P, 2 * QC], F32, tag="logp", name="log_ps")
                for h in range(HPC):
                    nc.tensor.matmul(
                        lp[:, h * QC:h * QC + N],
                        lhsT=qkv_sb[kb // 4][1][D * h:D * (h + 1),
                                                (kb % 4) * P:(kb % 4 + 1) * P],
                        rhs=qkv_sb[qc][0][D * h:D * (h + 1), lqsl],
                        start=True, stop=True)
                et = expp.tile([P, 2, QC], BF16, tag="et", name="exp_sb")
                nc.scalar.activation(
                    et[:, :, :N],
                    lp[:, :].rearrange("p (b n) -> p b n", b=2)[:, :, :N],
                    EXP)
                if diag:
                    nc.vector.tensor_mul(
                        et[:, :, 0:P], et[:, :, 0:P],
                        tri[:, None, :].to_broadcast((P, 2, P)))
                if len(pend) >= 3:
                    flush(pend.pop(0), False)
                pend.append((kb, et, qoff, N))
                if fillers and kb % 2 == 1:
                    fillers.pop(0)()
            for f in fillers:
                f()
            last_av = None
            while pend:
                last_av = flush(pend.pop(0), not pend)

            # normalize rows 0:64 by row 64 (exp sums), both heads into one
            # staging tile, then scatter stripes into the a2a input buffer
            g, slot = next((g, s) for g in range(NG)
                           for s in range(len(GROUPS[g])) if GROUPS[g][s] == qc)
            stage = stagep.tile([P, QC], BF16, tag="stage", name="stage")
            for h in range(HPC):
                rs = smallp.tile([1, QC], F32, tag="rs", name="rs")
                nc.vector.tensor_copy(rs[:, :], avp[h][D:D + 1, :])
                rr = smallp.tile([1, QC], F32, tag="rr", name="rr")
                nc.vector.reciprocal_approx_fast(rr[:, :], rs[:, :])
                rb = smallp.tile([D, QC], F32, tag="rb", name="rb")
                nc.gpsimd.partition_broadcast(rb[:, :], rr[:, :])
                nc.vector.tensor_mul(
                    stage[D * h:D * (h + 1), :], avp[h][0:D, :], rb[:, :])
            for h in range(HPC):
                nc.sync.dma_start(
                    a2a_in[g][:, :].rearrange("(j r) q -> r j q", r=P)
                    [D * h:D * (h + 1), :, slot * ST:(slot + 1) * ST],
                    stage[D * h:D * (h + 1), :]
                    .rearrange("p (j q) -> p j q", q=ST))
            if slot == len(GROUPS[g]) - 1:
                nc.gpsimd.collective_compute(
                    "AllToAll", mybir.AluOpType.bypass,
                    replica_groups=[list(range(NCORES))],
                    ins=[a2a_in[g][:, :].opt()],
                    outs=[a2a_out[g][:, :].opt()])
            return last_av

        def emit_proj(ph, anchor):
            # projection for stripe pair (2*ph, 2*ph+1): 128 output rows.
            # anchor: keep these instructions behind the attention stream in
            # the static schedule - the scheduler otherwise hoists them and
            # the collective-gated loads head-of-line block the PE/DMA FIFOs.
            def pin(inst):
                if anchor is not None:
                    add_dep_helper(inst.ins, anchor.ins, sync=False,
                                   reason="proj after attention")
                return inst
            # stripe s (0..3, out-row order) lives in group g at slot
            def stripe_src(s, k):
                acc = 0
                for g in range(NG):
                    if s < acc + len(GROUPS[g]):
                        slot = s - acc
                        return at_sb[g][:, k, slot * ST:(slot + 1) * ST]
                    acc += len(GROUPS[g])
            ob = outp.tile([P, E], F32, tag="ob", name="ob")
            for n in range(E // QC):
                nsl = slice(n * QC, (n + 1) * QC)
                pp = psmm.tile([P, QC], F32, tag="mmp", name="proj_ps")
                for k in range(KCH):
                    for half in range(2):
                        pin(nc.tensor.matmul(
                            pp[half * ST:(half + 1) * ST, :],
                            lhsT=stripe_src(2 * ph + half, k),
                            rhs=wp_sb[:, k, nsl],
                            start=(k == 0), stop=False,
                            tile_position=(0, half * ST)))
                pin(nc.tensor.matmul(
                    pp[:, :], lhsT=ones1[:, :], rhs=bp_sb[:, nsl],
                    start=False, stop=True))
                nc.vector.tensor_copy(ob[:, nsl], pp[:, :])
                nc.sync.dma_start(out[ph * P:(ph + 1) * P, nsl], ob[:, nsl])

        # ---- emission: interleave qkv chunks with attention so both PE
        # phases and the ACT exp stream overlap; attention runs 1,2,3,0 so
        # the final collective only waits on the shortest chunk (qc0).
        emit_qkv(0)
        for kb in range(0, 4):
            emit_vtrans(kb)
        emit_qkv(1)
        for kb in range(4, 8):
            emit_vtrans(kb)
        for k in range(KCH):
            eng = nc.sync if k % 2 == 0 else nc.scalar
            eng.dma_start(wp_sb[:, k, :], wpt[k * P:(k + 1) * P, :])
        nc.sync.dma_start(bp_sb[:, :], bp[:, :])
        f1 = [lambda m=m: emit_qkv_m(2, m) for m in range(3)] + \
             [lambda kb=kb: emit_vtrans(kb) for kb in range(8, 12)]
        emit_attn(1, f1)
        f2 = [lambda m=m: emit_qkv_m(3, m) for m in range(3)] + \
             [lambda kb=kb: emit_vtrans(kb) for kb in range(12, 16)]
        emit_attn(2, f2)
        emit_attn(3)
        last_av = emit_attn(0)
        for g in range(NG):
            d = nc.sync.dma_start(
                at_sb[g][:, :, :],
                a2a_out[g][:, :].rearrange("(kc p) q -> p kc q", p=P))
            add_dep_helper(d.ins, last_av.ins, sync=False,
                           reason="gathered loads after attention")
        for ph in range(2):
            emit_proj(ph, last_av)


_NC_CACHE = None


def _get_nc():
    global _NC_CACHE
    if _NC_CACHE is None:
        _NC_CACHE = build_nc()
    return _NC_CACHE


def make_in_maps(x, w_attn, b_attn, w_proj, b_proj):
    bf16 = ml_dtypes.bfloat16
    x = np.asarray(x, dtype=np.float32)
    w_attn = np.asarray(w_attn, dtype=np.float32)
    b_attn = np.asarray(b_attn, dtype=np.float32)
    w_proj = np.asarray(w_proj, dtype=np.float32)
    b_proj = np.asarray(b_proj, dtype=np.float32)

    xt = np.ascontiguousarray(x.T).astype(bf16)          # (E, S)
    wpt = np.ascontiguousarray(w_proj.T).astype(bf16)    # (E, E)
    bpa = np.ascontiguousarray(b_proj[None, :]).astype(bf16)
    scale = 1.0 / np.sqrt(D)

    in_maps = []
    for c in range(NCORES):
        rq = slice(F * c, F * (c + 1))
        rk = slice(E + F * c, E + F * (c + 1))
        rv = slice(2 * E + F * c, 2 * E + F * (c + 1))
        wqkv = np.ascontiguousarray(np.concatenate(
            [w_attn[rq] * scale, w_attn[rk], w_attn[rv]], axis=0).T)
        bq = np.stack([b_attn[rq] * scale, b_attn[rk], b_attn[rv]], axis=1)
        in_maps.append({
            "xt": xt,
            "wqkv": wqkv.astype(bf16),
            "bqkv": np.ascontiguousarray(bq, dtype=np.float32),
            "wpt": wpt,
            "bp": bpa,
        })
    return in_maps


def run(inputs, trace=False, **kw):
    from concourse.bass_utils import run_bass_kernel_spmd
    nc = _get_nc()
    in_maps = make_in_maps(**inputs)
    res = run_bass_kernel_spmd(nc, in_maps, core_ids=list(range(NCORES)),
                               trace=trace, **kw)
    # core j's out row blocks are stripes for qc = 3,2,1,0 in that order;
    # stripe qc covers global rows 512*qc + 64*j .. +64
    full = np.empty((S, E), dtype=np.float32)
    for j in range(NCORES):
        o = res.results[j]["out"]                        # (256, E)
        for blk, qc in enumerate([1, 2, 3, 0]):
            full[QC * qc + ST * j: QC * qc + ST * (j + 1), :] = \
                o[ST * blk: ST * (blk + 1), :]
    return full, res


def kernel(**inputs):
    full, _ = run(inputs, trace=False)
    return full
